# revision 10
# baseline (speedup 1.0000x reference)
"""CANLayer (two-edge-set multi-head cell attention + skip) on 8 TRN2 NeuronCores.

Self-contained: hardcodes shapes for N=50000 cells, E=800000 edges/set,
C_IN=128, HEADS=4, D_OUT=32.

Strategy:
 - Cells are 1D-partitioned across 8 cores (6272 aligned cells each); edges are
   routed to the core owning their target cell (host-side, part of sharding).
 - Each core redundantly computes per-node tables in DRAM:
     table[s][n] = [xm_s(n) as 128 bf16 | ss_s(n) as 4 f32 | pad]  (512B rows)
   where xm = x @ W_s and ss = x @ (W_s @ a_src_s) (attention source logit).
 - Edge phase: per 128-target-cell window, dma_gather pulls the 512B rows for
   each edge (int16 indices, split over two table halves); attention weights
   use the shift-free identity  softmax(LR(ss+sd)) == normalize over segment of
   exp(LR(ss+sd)), computed per edge with sd broadcast from the window's
   target cells via a one-hot^T matmul; aggregation is a one-hot matmul
   accumulated in PSUM (cells x [128 msg | 4 denom]).
 - Output: relu(agg_low/denom_low + agg_up/denom_up + EPS*(x@W_skip+b_skip)).
"""
import sys
sys.path.insert(0, "/opt/trn_rl_repo")

import os

import numpy as np
import ml_dtypes

import concourse.bass as bass
import concourse.mybir as mybir
import concourse.tile as tile
from concourse import bacc
from concourse.bass_utils import run_bass_kernel_spmd

BF16 = mybir.dt.bfloat16
F32 = mybir.dt.float32
I16 = mybir.dt.int16

N_CELLS = 50000
N_EDGES = 800000
C_IN = 128
HEADS = 4
D_OUT = 32
HD = HEADS * D_OUT          # 128
EPS = 1.0 + 1e-6
NEG_SLOPE = 0.01

N_CORES = 8
CPC = 6272                  # cells per core (49 * 128), last core ragged
NW = 49                     # windows (128 cells) per core
NT = 391                    # node tiles over padded 50048 cells
NPAD = NT * 128             # 50048
TAB_ROWS = NPAD             # table rows
HALF = 25024                # int16-index table split
BPH = 10                    # blocks (128 edges) per half per window
BPW = 2 * BPH               # 20 blocks per window
SLOTS_H = BPH * 128         # 1280 slots per half
SLOTS_W = BPW * 128         # 2560 slots per window
RCOL = 256                  # table row cols (bf16) = 512B
XCOL = 128                  # xm cols
TRACE = False
NW_RUN = int(os.environ.get("KERNEL_NW", NW))
SIM_SAFE = os.environ.get("KERNEL_SIM_SAFE", "0") == "1"
STAGE = int(os.environ.get("KERNEL_STAGE", "3"))

_CACHED = {}


def _build_nc():
    nc = bacc.Bacc(None)

    # ---- per-core inputs ----
    x_bf = nc.declare_dram_parameter("x_bf", [NPAD, C_IN], BF16, isOutput=False)
    x_own = nc.declare_dram_parameter("x_own", [CPC, C_IN], BF16, isOutput=False)
    w_all = nc.declare_dram_parameter("w_all", [C_IN, 264], BF16, isOutput=False)
    w_own = nc.declare_dram_parameter("w_own", [C_IN, 136], BF16, isOutput=False)
    b_rep = nc.declare_dram_parameter("b_rep", [128, 128], F32, isOutput=False)
    iota_in = nc.declare_dram_parameter("iota", [128, 128], BF16, isOutput=False)
    ident_in = nc.declare_dram_parameter("ident", [128, 128], BF16, isOutput=False)
    idx16 = [
        nc.declare_dram_parameter(f"idx16_{s}", [128, NW * 2 * (SLOTS_H // 16)], I16,
                                  isOutput=False)
        for s in range(2)
    ]
    tgtl = [
        nc.declare_dram_parameter(f"tgtl_{s}", [128, NW * BPW], F32, isOutput=False)
        for s in range(2)
    ]
    cnts = [
        nc.declare_dram_parameter(f"cnt_{s}", [1, NW * 2], mybir.dt.int32,
                                  isOutput=False)
        for s in range(2)
    ]
    out = nc.declare_dram_parameter("out", [CPC, HD], F32, isOutput=True)

    # ---- DRAM internals ----
    tables = [nc.dram_tensor(f"table_{s}", [TAB_ROWS, RCOL], BF16) for s in range(2)]

    IPH = SLOTS_H // 16      # idx16 cols per half (80)

    with tile.TileContext(nc) as tc:
        # ---------- persistent SBUF ----------
        with tc.tile_pool(name="persist", bufs=1) as pers:
            t_iota = pers.tile([128, 128], BF16)
            t_ident = pers.tile([128, 128], BF16)
            t_brep = pers.tile([128, 128], F32)
            t_idx = [pers.tile([128, NW * 2 * IPH], I16, tag=f"idx{s}", name=f"tidx{s}") for s in range(2)]
            t_tgtl = [pers.tile([128, NW * BPW], F32, tag=f"tgtl{s}", name=f"ttgtl{s}") for s in range(2)]
            t_sdw = [pers.tile([128, NW * HEADS], BF16, tag=f"sdw{s}", name=f"tsdw{s}") for s in range(2)]
            t_skip = pers.tile([128, NW * 128], F32)
            t_cnt = [pers.tile([1, NW * 2], mybir.dt.int32, tag=f"cnt{s}",
                               name=f"tcnt{s}") for s in range(2)]

            nc.sync.dma_start(out=t_iota[:], in_=iota_in[:])
            nc.sync.dma_start(out=t_ident[:], in_=ident_in[:])
            nc.sync.dma_start(out=t_brep[:], in_=b_rep[:])
            for s in range(2):
                nc.sync.dma_start(out=t_idx[s][:], in_=idx16[s][:])
                nc.sync.dma_start(out=t_tgtl[s][:], in_=tgtl[s][:])
                nc.sync.dma_start(out=t_cnt[s][:], in_=cnts[s][:])

            # ---------- node phase ----------
            with tc.tile_pool(name="node_sb", bufs=1) as nsb, \
                 tc.tile_pool(name="node_stage", bufs=3) as nst, \
                 tc.tile_pool(name="node_ps", bufs=4, space="PSUM") as nps:
                t_wall = nsb.tile([128, 264], BF16)
                t_wown = nsb.tile([128, 136], BF16)
                nc.sync.dma_start(out=t_wall[:], in_=w_all[:])
                nc.sync.dma_start(out=t_wown[:], in_=w_own[:])

                t_xT = nsb.tile([128, NPAD], BF16)
                CH = 3072  # transpose-dma chunk (rows, multiple of 128)
                for c0 in range(0, NPAD, CH):
                    ce = min(CH, NPAD - c0)
                    nc.sync.dma_start(out=t_xT[:, c0:c0 + ce],
                                      in_=x_bf[c0:c0 + ce, :], transpose=True)

                for t in range(NT):
                    ps = nps.tile([128, 264], F32, tag="nps")
                    nc.tensor.matmul(ps[:], t_xT[:, t * 128:(t + 1) * 128],
                                     t_wall[:], start=True, stop=True)
                    for s in range(2):
                        stg = nst.tile([128, RCOL], BF16, tag=f"stg{s}", name=f"stg{s}")
                        if SIM_SAFE or t < 3:
                            nc.gpsimd.memset(stg[:], 0)
                        nc.scalar.copy(out=stg[:, 0:XCOL],
                                       in_=ps[:, 0 + 128 * s:128 + 128 * s])
                        ss_view = stg[:, XCOL:XCOL + 8].bitcast(F32)
                        nc.vector.tensor_copy(out=ss_view,
                                              in_=ps[:, 256 + 4 * s:256 + 4 * s + 4])
                        nc.sync.dma_start(out=tables[s][t * 128:(t + 1) * 128, :],
                                          in_=stg[:])

                # own pass: sd + skip for this core's cells
                t_xoT = nsb.tile([128, CPC], BF16)
                for c0 in range(0, CPC, CH):
                    ce = min(CH, CPC - c0)
                    nc.sync.dma_start(out=t_xoT[:, c0:c0 + ce],
                                      in_=x_own[c0:c0 + ce, :], transpose=True)
                for t in range(NW):
                    ps = nps.tile([128, 136], F32, tag="ops")
                    nc.tensor.matmul(ps[:], t_xoT[:, t * 128:(t + 1) * 128],
                                     t_wown[:], start=True, stop=True)
                    for s in range(2):
                        nc.vector.tensor_copy(
                            out=t_sdw[s][:, t * HEADS:(t + 1) * HEADS],
                            in_=ps[:, 4 * s:4 * s + 4])
                    # skip with bias
                    nc.vector.scalar_tensor_tensor(
                        out=t_skip[:, t * 128:(t + 1) * 128],
                        in0=ps[:, 8:136], scalar=0.0,
                        in1=t_brep[:],
                        op0=mybir.AluOpType.add, op1=mybir.AluOpType.add)

            # ---------- edge phase ----------
            with tc.tile_pool(name="eg", bufs=2) as egp, \
                 tc.tile_pool(name="ea", bufs=2) as eap, \
                 tc.tile_pool(name="esm", bufs=2) as esm, \
                 tc.tile_pool(name="eat", bufs=4) as eat, \
                 tc.tile_pool(name="eps", bufs=2, space="PSUM") as epp, \
                 tc.tile_pool(name="epsb", bufs=2, space="PSUM") as epb, \
                 tc.tile_pool(name="ecmb", bufs=2) as ecmb:
                for w in range(NW_RUN):
                    psA = [None, None]
                    for s in range(2 if STAGE >= 1 else 0):
                        G = egp.tile([128, BPW, RCOL], BF16, tag="G")
                        if SIM_SAFE or w == 0:
                            nc.gpsimd.memset(G[:], 0)
                        for half in range(2):
                            nreg = nc.gpsimd.value_load(
                                t_cnt[s][0:1, w * 2 + half:w * 2 + half + 1])
                            nc.gpsimd.dma_gather(
                                out_ap=G[:, half * BPH:(half + 1) * BPH, :],
                                in_ap=tables[s][half * HALF:half * HALF + HALF, :],
                                idxs_ap=t_idx[s][:, (w * 2 + half) * IPH:
                                                 (w * 2 + half + 1) * IPH],
                                num_idxs=SLOTS_H,
                                num_idxs_reg=nreg,
                                elem_size=RCOL,
                                single_packet=False,
                            )
                        if STAGE < 2:
                            continue
                        A = eap.tile([128, BPW, 128], BF16, tag="A")
                        sd_ps = epb.tile([128, BPW * HEADS], F32, tag="sdps")
                        for b in range(BPW):
                            nc.vector.tensor_scalar(
                                out=A[:, b, :], in0=t_iota[:],
                                scalar1=t_tgtl[s][:, w * BPW + b:w * BPW + b + 1],
                                scalar2=None, op0=mybir.AluOpType.is_equal)
                        for b in range(BPW):
                            atp = epb.tile([128, 128], BF16, tag="atp")
                            nc.tensor.transpose(out=atp[:], in_=A[:, b, :],
                                                identity=t_ident[:])
                            at_sb = eat.tile([128, 128], BF16, tag="atsb")
                            # split PSUM->SBUF copies across DVE and Act
                            if b % 5 < 3:
                                nc.vector.tensor_copy(out=at_sb[:], in_=atp[:])
                            else:
                                nc.scalar.copy(out=at_sb[:], in_=atp[:])
                            nc.tensor.matmul(
                                sd_ps[:, b * HEADS:(b + 1) * HEADS],
                                at_sb[:],
                                t_sdw[s][:, w * HEADS:(w + 1) * HEADS],
                                start=True, stop=True)
                        # window-batched softmax weights (single add: ss + sd)
                        alpha = esm.tile([128, BPW * HEADS], F32, tag="alpha")
                        nc.vector.tensor_tensor(
                            out=alpha[:].rearrange("p (b h) -> p b h", h=HEADS),
                            in0=G[:, :, XCOL:XCOL + 8].bitcast(F32),
                            in1=sd_ps[:].rearrange("p (b h) -> p b h", h=HEADS),
                            op=mybir.AluOpType.add)
                        lr = esm.tile([128, BPW * HEADS], F32, tag="lr")
                        nc.vector.scalar_tensor_tensor(
                            out=lr[:], in0=alpha[:], scalar=NEG_SLOPE,
                            in1=alpha[:],
                            op0=mybir.AluOpType.mult, op1=mybir.AluOpType.max)
                        e_w = esm.tile([128, BPW * HEADS], BF16, tag="ew")
                        nc.scalar.activation(out=e_w[:], in_=lr[:],
                                             func=mybir.ActivationFunctionType.Exp)
                        if STAGE < 3:
                            continue
                        pme = egp.tile([128, BPW, 132], BF16, tag="pme")
                        nc.vector.tensor_copy(
                            out=pme[:, :, 128:132],
                            in_=e_w[:].rearrange("p (b h) -> p b h", h=HEADS))
                        # one batched multiply for the whole window: xm columns
                        # are (d,h)-interleaved so every AP is packed bf16
                        ew_ap = e_w[:]
                        ew3 = bass.AP(ew_ap.tensor, ew_ap.offset,
                                      [ew_ap.ap[0], [HEADS, BPW], [0, D_OUT],
                                       [1, HEADS]])
                        nc.vector.tensor_tensor(
                            out=pme[:, :, 0:XCOL], in0=G[:, :, 0:XCOL],
                            in1=ew3, op=mybir.AluOpType.mult)
                        ps_agg = epp.tile([128, 132], F32, tag=f"agg{s}")
                        for b in range(BPW):
                            nc.tensor.matmul(ps_agg[:], A[:, b, :], pme[:, b, :],
                                             start=(b == 0), stop=(b == BPW - 1))
                        psA[s] = ps_agg

                    # ---- combine window ----
                    if STAGE < 3:
                        outt0 = ecmb.tile([128, 128], F32, tag="outt")
                        nc.vector.tensor_scalar_max(
                            outt0[:], t_skip[:, w * 128:(w + 1) * 128], 0.0)
                        nc.sync.dma_start(out=out[w * 128:(w + 1) * 128, :],
                                          in_=outt0[:])
                        continue
                    rec = [None, None]
                    for s in range(2):
                        dn = ecmb.tile([128, HEADS], F32, tag=f"dn{s}")
                        nc.vector.tensor_scalar_add(dn[:], psA[s][:, 128:132], 1e-16)
                        rc = ecmb.tile([128, HEADS], F32, tag=f"rc{s}")
                        nc.vector.reciprocal(out=rc[:], in_=dn[:])
                        rec[s] = rc
                    acc = ecmb.tile([128, 128], F32, tag="acc")
                    r0 = rec[0][:]
                    r0b = bass.AP(r0.tensor, r0.offset,
                                  [r0.ap[0], [0, D_OUT], [1, HEADS]])
                    nc.vector.tensor_tensor(
                        out=acc[:], in0=psA[0][:, 0:128],
                        in1=r0b, op=mybir.AluOpType.mult)
                    acc2 = ecmb.tile([128, 128], F32, tag="acc2")
                    r1 = rec[1][:]
                    r1b = bass.AP(r1.tensor, r1.offset,
                                  [r1.ap[0], [0, D_OUT], [1, HEADS]])
                    nc.vector.tensor_tensor(
                        out=acc2[:], in0=psA[1][:, 0:128],
                        in1=r1b, op=mybir.AluOpType.mult)
                    nc.vector.tensor_add(out=acc[:], in0=acc[:], in1=acc2[:])
                    nc.vector.tensor_add(out=acc[:], in0=acc[:],
                                         in1=t_skip[:, w * 128:(w + 1) * 128])
                    outt = ecmb.tile([128, 128], F32, tag="outt")
                    # un-permute (d,h) columns back to (h,d) via strided write
                    ot = outt[:]
                    otperm = bass.AP(ot.tensor, ot.offset,
                                     [ot.ap[0], [1, D_OUT], [D_OUT, HEADS]])
                    nc.vector.tensor_scalar_max(otperm, acc[:], 0.0)
                    nc.sync.dma_start(out=out[w * 128:(w + 1) * 128, :],
                                      in_=outt[:])

    nc.finalize()
    return nc


def _fold(W, a):
    # W: [C_IN, HD] f32, a: [HEADS, D_OUT] -> [C_IN, HEADS]
    return np.einsum("chd,hd->ch",
                     W.astype(np.float64).reshape(C_IN, HEADS, D_OUT),
                     a.astype(np.float64)).astype(np.float32)


# (d,h)-interleaved column permutation: new col d*HEADS+h <- old col h*D_OUT+d
_PERM = np.array([h * D_OUT + d for d in range(D_OUT) for h in range(HEADS)],
                 dtype=np.int64)


def _edge_arrays(tgt, src):
    """Per-core idx16 / tgtl / count arrays for one edge set."""
    idx_all = np.full((N_CORES, 128, NW * 2 * (SLOTS_H // 16)), -1, np.int16)
    tgl_all = np.full((N_CORES, 128, NW * BPW), -1.0, np.float32)
    cnt_all = np.zeros((N_CORES, 1, NW * 2), np.int32)
    order = np.argsort(tgt, kind="stable")
    tgt_s = tgt[order]
    src_s = src[order]
    core_of = tgt_s // CPC
    core_of = np.minimum(core_of, N_CORES - 1)
    for c in range(N_CORES):
        m = core_of == c
        tc_, sc_ = tgt_s[m] - c * CPC, src_s[m]
        wi = tc_ // 128
        tl = tc_ - wi * 128
        for w in range(NW):
            mw = wi == w
            tw, sw = tl[mw], sc_[mw]
            for half in range(2):
                if half == 0:
                    mh = sw < HALF
                    sidx = sw[mh]
                else:
                    mh = sw >= HALF
                    sidx = sw[mh] - HALF
                th = tw[mh]
                n = len(sidx)
                if n > SLOTS_H:
                    raise OverflowError("half-window overflow")
                flat_i = np.full(SLOTS_H, -1, np.int16)
                flat_i[:n] = sidx.astype(np.int16)
                wrap = flat_i.reshape(SLOTS_H // 16, 16).T  # [16, IPH]
                col0 = (w * 2 + half) * (SLOTS_H // 16)
                idx_all[c, :, col0:col0 + SLOTS_H // 16] = np.tile(wrap, (8, 1))
                # tgtl: slot (b,p): block b within window = half*BPH + i//128
                tl_flat = np.full(SLOTS_H, -1.0, np.float32)
                tl_flat[:n] = th.astype(np.float32)
                blk = tl_flat.reshape(BPH, 128)  # [b, p]
                b0 = w * BPW + half * BPH
                tgl_all[c, :, b0:b0 + BPH] = blk.T
                cnt_all[c, 0, w * 2 + half] = n
    return idx_all, tgl_all, cnt_all


def kernel(x, lower_tgt, lower_src, upper_tgt, upper_src,
           W_low, a_src_low, a_dst_low, W_up, a_src_up, a_dst_up,
           W_skip, b_skip):
    if "nc" not in _CACHED:
        _CACHED["nc"] = _build_nc()
    nc = _CACHED["nc"]

    x = np.asarray(x, np.float32)
    x_bf_full = np.zeros((NPAD, C_IN), ml_dtypes.bfloat16)
    x_bf_full[:N_CELLS] = x.astype(ml_dtypes.bfloat16)

    w_all = np.zeros((C_IN, 264), np.float32)
    w_all[:, 0:128] = W_low[:, _PERM]
    w_all[:, 128:256] = W_up[:, _PERM]
    w_all[:, 256:260] = _fold(W_low, a_src_low)
    w_all[:, 260:264] = _fold(W_up, a_src_up)
    w_all = w_all.astype(ml_dtypes.bfloat16)

    w_own = np.zeros((C_IN, 136), np.float32)
    w_own[:, 0:4] = _fold(W_low, a_dst_low)
    w_own[:, 4:8] = _fold(W_up, a_dst_up)
    w_own[:, 8:136] = EPS * W_skip[:, _PERM]
    w_own = w_own.astype(ml_dtypes.bfloat16)

    b_rep = np.broadcast_to((EPS * b_skip).astype(np.float32)[_PERM],
                            (128, 128)).copy()
    iota = np.broadcast_to(np.arange(128, dtype=ml_dtypes.bfloat16),
                           (128, 128)).copy()
    ident = np.eye(128, dtype=ml_dtypes.bfloat16)

    idx0, tgl0, cnt0 = _edge_arrays(np.asarray(lower_tgt), np.asarray(lower_src))
    idx1, tgl1, cnt1 = _edge_arrays(np.asarray(upper_tgt), np.asarray(upper_src))

    in_maps = []
    for c in range(N_CORES):
        xo = np.zeros((CPC, C_IN), ml_dtypes.bfloat16)
        lo, hi = c * CPC, min((c + 1) * CPC, N_CELLS)
        if c == N_CORES - 1:
            hi = N_CELLS
        xo[:hi - lo] = x[lo:hi].astype(ml_dtypes.bfloat16)
        in_maps.append(dict(
            x_bf=x_bf_full, x_own=xo, w_all=w_all, w_own=w_own, b_rep=b_rep,
            iota=iota, ident=ident,
            idx16_0=idx0[c], idx16_1=idx1[c], tgtl_0=tgl0[c], tgtl_1=tgl1[c],
            cnt_0=cnt0[c], cnt_1=cnt1[c],
        ))

    res = run_bass_kernel_spmd(nc, in_maps, core_ids=list(range(N_CORES)),
                               trace=TRACE)
    outs = []
    for c in range(N_CORES):
        lo = c * CPC
        hi = min(lo + CPC, N_CELLS)
        outs.append(res.results[c]["out"][:hi - lo])
    full = np.concatenate(outs, axis=0)
    if TRACE:
        kernel.last_exec_ns = res.exec_time_ns
        kernel.last_results = res
    return full.astype(np.float32)



# revision 36
# speedup vs baseline: 80.1321x; 80.1321x over previous
"""CANLayer (two-edge-set multi-head cell attention + skip) on 8 TRN2 NeuronCores.

Self-contained: hardcodes shapes for N=50000 cells, E=800000 edges/set,
C_IN=128, HEADS=4, D_OUT=32.

Strategy (v2):
 - Cells are 1D-partitioned across 8 cores (6272 aligned cells each); edges are
   routed to the core owning their target cell (host-side, part of sharding).
 - Each core redundantly computes a per-node table in DRAM (one merged table,
   1024B rows):
     row[n] = [xm_l_perm(128 bf16) | ss_l(4 bf16) | 0*124 |
               xm_u_perm(128 bf16) | ss_u(4 bf16) | 0*124]
   where xm = x @ W_s with (d,h)-interleaved columns and ss = x @ (W_s @ a_src)
   (attention source logit).  The node matmul writes bf16 straight to PSUM and
   one DMA per 128-cell tile copies PSUM -> DRAM (no staging copies).
 - Edge phase: per 128-target-cell window, dma_gather pulls 512B row halves for
   each edge (int16 indices, split over two table halves); attention weights
   use the shift-free identity  softmax(LR(ss+sd)) == normalize over segment of
   exp(LR(ss+sd)), with sd broadcast from the window's target cells via a
   one-hot^T matmul (transposes batched through PSUM in groups, single DVE
   copy); message weighting is one batched multiply per window (packed bf16
   APs via the (d,h) interleave); aggregation is a one-hot matmul accumulated
   in PSUM (cells x [128 msg | 4 denom]).
 - Output: relu(agg_low/denom_low + agg_up/denom_up + EPS*(x@W_skip+b_skip)),
   columns un-permuted during the final strided write.
"""
import sys
sys.path.insert(0, "/opt/trn_rl_repo")

import os

import numpy as np
import ml_dtypes

import concourse.bass as bass
import concourse.mybir as mybir
import concourse.tile as tile
from concourse import bacc
from concourse.bass_utils import run_bass_kernel_spmd

BF16 = mybir.dt.bfloat16
F32 = mybir.dt.float32
I16 = mybir.dt.int16

N_CELLS = 50000
N_EDGES = 800000
C_IN = 128
HEADS = 4
D_OUT = 32
HD = HEADS * D_OUT          # 128
EPS = 1.0 + 1e-6
NEG_SLOPE = 0.01

N_CORES = 8
CPC = 6272                  # cells per core (49 * 128), last core ragged
NW = 49                     # windows (128 cells) per core
NT = 391                    # node tiles over padded 50048 cells
NPAD = NT * 128             # 50048
TAB_ROWS = NPAD             # table rows
HALF = 25024                # int16-index table split
RROW = 512                  # merged table row (bf16 elems) = 1024B
RCOL = 256                  # per-set row half (bf16 elems) = 512B
XCOL = 128                  # xm cols
TRACE = False
NW_RUN = int(os.environ.get("KERNEL_NW", NW))
SIM_SAFE = os.environ.get("KERNEL_SIM_SAFE", "0") == "1"
STAGE = int(os.environ.get("KERNEL_STAGE", "3"))

_CACHED = {}


def _build_nc(bph):
    bpw = 2 * bph            # blocks (128 edges) per window
    slots_h = bph * 128      # slots per half per window
    iph = slots_h // 16      # idx16 cols per half

    nc = bacc.Bacc(None)

    # ---- per-core inputs ----
    xT_bf = nc.declare_dram_parameter("xT_bf", [C_IN, NPAD], BF16, isOutput=False)
    xT_own = nc.declare_dram_parameter("xT_own", [C_IN, CPC], BF16, isOutput=False)
    w_all = nc.declare_dram_parameter("w_all", [C_IN, 264], BF16, isOutput=False)
    w_own = nc.declare_dram_parameter("w_own", [C_IN, 136], BF16, isOutput=False)
    b_rep = nc.declare_dram_parameter("b_rep", [128, 128], F32, isOutput=False)
    iota_in = nc.declare_dram_parameter("iota", [128, 128], BF16, isOutput=False)
    ident_in = nc.declare_dram_parameter("ident", [128, 128], BF16, isOutput=False)
    idx16 = [
        nc.declare_dram_parameter(f"idx16_{s}", [128, NW * 2 * iph], I16,
                                  isOutput=False)
        for s in range(2)
    ]
    tgtl = [
        nc.declare_dram_parameter(f"tgtl_{s}", [128, NW * bpw], F32, isOutput=False)
        for s in range(2)
    ]
    cnts = [
        nc.declare_dram_parameter(f"cnt_{s}", [1, NW * 2], mybir.dt.int32,
                                  isOutput=False)
        for s in range(2)
    ]
    out = nc.declare_dram_parameter("out", [CPC, HD], F32, isOutput=True)

    # ---- DRAM internals ----
    mtable = nc.dram_tensor("mtable", [TAB_ROWS, RROW], BF16)

    with tile.TileContext(nc) as tc:
        # ---------- persistent SBUF ----------
        with tc.tile_pool(name="persist", bufs=1) as pers:
            t_iota = pers.tile([128, 128], BF16)
            t_ident = pers.tile([128, 128], BF16)
            t_brep = pers.tile([128, 128], F32)
            t_idx = [pers.tile([128, NW * 2 * iph], I16, tag=f"idx{s}", name=f"tidx{s}") for s in range(2)]
            t_tgtl = [pers.tile([128, NW * bpw], F32, tag=f"tgtl{s}", name=f"ttgtl{s}") for s in range(2)]
            t_sdw = [pers.tile([128, NW * HEADS], BF16, tag=f"sdw{s}", name=f"tsdw{s}") for s in range(2)]
            t_skip = pers.tile([128, NW * 128], F32)
            t_cnt = [pers.tile([1, NW * 2], mybir.dt.int32, tag=f"cnt{s}",
                               name=f"tcnt{s}") for s in range(2)]

            nc.sync.dma_start(out=t_iota[:], in_=iota_in[:])
            nc.sync.dma_start(out=t_ident[:], in_=ident_in[:])
            nc.sync.dma_start(out=t_brep[:], in_=b_rep[:])
            for s in range(2):
                nc.sync.dma_start(out=t_idx[s][:], in_=idx16[s][:])
                nc.sync.dma_start(out=t_tgtl[s][:], in_=tgtl[s][:])
                nc.sync.dma_start(out=t_cnt[s][:], in_=cnts[s][:])

            # ---------- node phase ----------
            with tc.tile_pool(name="node_sb", bufs=1) as nsb, \
                 tc.tile_pool(name="node_stage", bufs=3) as nst, \
                 tc.tile_pool(name="node_ps", bufs=4, space="PSUM") as nps:
                t_wall = nsb.tile([128, 264], BF16)
                t_wown = nsb.tile([128, 136], BF16)
                nc.sync.dma_start(out=t_wall[:], in_=w_all[:])
                nc.sync.dma_start(out=t_wown[:], in_=w_own[:])

                t_xT = nsb.tile([128, NPAD], BF16)
                nc.sync.dma_start(out=t_xT[:], in_=xT_bf[:])

                # ps = [row_l(132) | row_u(132)] f32; two staged copies build
                # the merged (zero-padded) 1024B row; one batched DMA per
                # TB tiles (HWDGE/SP-SEQ cost is per-instruction)
                TB = 8
                for t0 in range(0, NT, TB):
                    tn = min(TB, NT - t0)
                    stg = nst.tile([128, TB, RROW], BF16, tag="stg")
                    if SIM_SAFE or t0 < 3 * TB:
                        nc.gpsimd.memset(stg[:], 0)
                    for tt in range(tn):
                        t = t0 + tt
                        ps = nps.tile([128, 264], F32, tag="nps")
                        nc.tensor.matmul(ps[:], t_xT[:, t * 128:(t + 1) * 128],
                                         t_wall[:], start=True, stop=True)
                        nc.scalar.copy(out=stg[:, tt, 0:132], in_=ps[:, 0:132])
                        nc.vector.tensor_copy(out=stg[:, tt, RCOL:RCOL + 132],
                                              in_=ps[:, 132:264])
                    mt = mtable[:]
                    mrows = bass.AP(mt.tensor, t0 * 128 * RROW,
                                    [[RROW, 128], [128 * RROW, tn], [1, RROW]])
                    nc.sync.dma_start(out=mrows, in_=stg[:, 0:tn, :])

                # own pass: sd + skip for this core's cells
                t_xoT = nsb.tile([128, CPC], BF16)
                nc.sync.dma_start(out=t_xoT[:], in_=xT_own[:])
                for t in range(NW):
                    ps = nps.tile([128, 136], F32, tag="ops")
                    nc.tensor.matmul(ps[:], t_xoT[:, t * 128:(t + 1) * 128],
                                     t_wown[:], start=True, stop=True)
                    for s in range(2):
                        nc.vector.tensor_copy(
                            out=t_sdw[s][:, t * HEADS:(t + 1) * HEADS],
                            in_=ps[:, 4 * s:4 * s + 4])
                    # skip with bias
                    nc.vector.scalar_tensor_tensor(
                        out=t_skip[:, t * 128:(t + 1) * 128],
                        in0=ps[:, 8:136], scalar=0.0,
                        in1=t_brep[:],
                        op0=mybir.AluOpType.add, op1=mybir.AluOpType.add)

            # ---------- edge phase ----------
            GH = (bpw + 1) // 2  # transpose group size (blocks per PSUM group)
            with tc.tile_pool(name="eg", bufs=2) as egp, \
                 tc.tile_pool(name="ea", bufs=2) as eap, \
                 tc.tile_pool(name="esm", bufs=2) as esm, \
                 tc.tile_pool(name="eat", bufs=2) as eat, \
                 tc.tile_pool(name="eps", bufs=2, space="PSUM") as epp, \
                 tc.tile_pool(name="epsb", bufs=2, space="PSUM") as epb, \
                 tc.tile_pool(name="epst", bufs=2, space="PSUM") as ept, \
                 tc.tile_pool(name="ecmb", bufs=2) as ecmb:
                for w in range(NW_RUN):
                    ps_agg = None
                    if STAGE >= 3:
                        ps_agg = epp.tile([128, 2, 132], F32, tag="agg")
                    for s in range(2 if STAGE >= 1 else 0):
                        G = egp.tile([128, bpw, RCOL], BF16, tag="G")
                        if SIM_SAFE or w == 0:
                            nc.gpsimd.memset(G[:], 0)
                        for half in range(2):
                            nreg = nc.gpsimd.value_load(
                                t_cnt[s][0:1, w * 2 + half:w * 2 + half + 1])
                            nc.gpsimd.dma_gather(
                                out_ap=G[:, half * bph:(half + 1) * bph, :],
                                in_ap=mtable[half * HALF:half * HALF + HALF,
                                             s * RCOL:(s + 1) * RCOL],
                                idxs_ap=t_idx[s][:, (w * 2 + half) * iph:
                                                 (w * 2 + half + 1) * iph],
                                num_idxs=slots_h,
                                num_idxs_reg=nreg,
                                elem_size=RCOL,
                                elem_step=RROW,
                                single_packet=False,
                            )
                        if STAGE < 2:
                            continue
                        A = eap.tile([128, bpw, 128], BF16, tag="A")
                        sd_ps = epb.tile([128, bpw * HEADS], F32, tag="sdps")
                        for b in range(bpw):
                            nc.vector.tensor_scalar(
                                out=A[:, b, :], in0=t_iota[:],
                                scalar1=t_tgtl[s][:, w * bpw + b:w * bpw + b + 1],
                                scalar2=None, op0=mybir.AluOpType.is_equal)
                        # transpose A blocks through PSUM in two groups with a
                        # single Act copy per group
                        for g in range(2):
                            b0, b1 = g * GH, min((g + 1) * GH, bpw)
                            atp = ept.tile([128, GH, 128], BF16, tag="atp")
                            for j in range(b1 - b0):
                                nc.tensor.transpose(out=atp[:, j, :],
                                                    in_=A[:, b0 + j, :],
                                                    identity=t_ident[:])
                            at_sb = eat.tile([128, GH, 128], BF16, tag="atsb")
                            nc.scalar.copy(out=at_sb[:], in_=atp[:])
                            for j in range(b1 - b0):
                                b = b0 + j
                                nc.tensor.matmul(
                                    sd_ps[:, b * HEADS:(b + 1) * HEADS],
                                    at_sb[:, j, :],
                                    t_sdw[s][:, w * HEADS:(w + 1) * HEADS],
                                    start=True, stop=True)
                        # window-batched softmax weights (ss is bf16 in-row)
                        alpha = esm.tile([128, bpw * HEADS], F32, tag="alpha")
                        nc.vector.tensor_tensor(
                            out=alpha[:].rearrange("p (b h) -> p b h", h=HEADS),
                            in0=G[:, :, XCOL:XCOL + HEADS],
                            in1=sd_ps[:].rearrange("p (b h) -> p b h", h=HEADS),
                            op=mybir.AluOpType.add)
                        lr = esm.tile([128, bpw * HEADS], F32, tag="lr")
                        nc.vector.scalar_tensor_tensor(
                            out=lr[:], in0=alpha[:], scalar=NEG_SLOPE,
                            in1=alpha[:],
                            op0=mybir.AluOpType.mult, op1=mybir.AluOpType.max)
                        e_w = esm.tile([128, bpw * HEADS], BF16, tag="ew")
                        nc.scalar.activation(out=e_w[:], in_=lr[:],
                                             func=mybir.ActivationFunctionType.Exp)
                        if STAGE < 3:
                            continue
                        pme = egp.tile([128, bpw, 132], BF16, tag="pme")
                        nc.vector.tensor_copy(
                            out=pme[:, :, 128:132],
                            in_=e_w[:].rearrange("p (b h) -> p b h", h=HEADS))
                        # one batched multiply for the whole window: xm columns
                        # are (d,h)-interleaved so every AP is packed bf16
                        ew_ap = e_w[:]
                        ew3 = bass.AP(ew_ap.tensor, ew_ap.offset,
                                      [ew_ap.ap[0], [HEADS, bpw], [0, D_OUT],
                                       [1, HEADS]])
                        nc.vector.tensor_tensor(
                            out=pme[:, :, 0:XCOL], in0=G[:, :, 0:XCOL],
                            in1=ew3, op=mybir.AluOpType.mult)
                        for b in range(bpw):
                            nc.tensor.matmul(ps_agg[:, s, :], A[:, b, :],
                                             pme[:, b, :],
                                             start=(b == 0), stop=(b == bpw - 1))

                    # ---- combine window ----
                    if STAGE < 3:
                        outt0 = ecmb.tile([128, 128], F32, tag="outt")
                        nc.vector.tensor_scalar_max(
                            outt0[:], t_skip[:, w * 128:(w + 1) * 128], 0.0)
                        nc.sync.dma_start(out=out[w * 128:(w + 1) * 128, :],
                                          in_=outt0[:])
                        continue
                    # copy PSUM agg to SBUF once (Act), then combine on Pool
                    agg_sb = ecmb.tile([128, 2, 132], F32, tag="aggsb")
                    nc.scalar.copy(out=agg_sb[:], in_=ps_agg[:])
                    rec = [None, None]
                    for s in range(2):
                        dn = ecmb.tile([128, HEADS], F32, tag=f"dn{s}")
                        nc.vector.tensor_scalar_add(dn[:], agg_sb[:, s, 128:132],
                                                    1e-16)
                        rc = ecmb.tile([128, HEADS], F32, tag=f"rc{s}")
                        nc.vector.reciprocal(out=rc[:], in_=dn[:])
                        rec[s] = rc
                    acc = ecmb.tile([128, 128], F32, tag="acc")
                    r0 = rec[0][:]
                    r0b = bass.AP(r0.tensor, r0.offset,
                                  [r0.ap[0], [0, D_OUT], [1, HEADS]])
                    nc.gpsimd.tensor_tensor(
                        out=acc[:], in0=agg_sb[:, 0, 0:128],
                        in1=r0b, op=mybir.AluOpType.mult)
                    acc2 = ecmb.tile([128, 128], F32, tag="acc2")
                    r1 = rec[1][:]
                    r1b = bass.AP(r1.tensor, r1.offset,
                                  [r1.ap[0], [0, D_OUT], [1, HEADS]])
                    nc.gpsimd.tensor_tensor(
                        out=acc2[:], in0=agg_sb[:, 1, 0:128],
                        in1=r1b, op=mybir.AluOpType.mult)
                    nc.gpsimd.tensor_add(out=acc[:], in0=acc[:], in1=acc2[:])
                    nc.gpsimd.tensor_add(out=acc[:], in0=acc[:],
                                         in1=t_skip[:, w * 128:(w + 1) * 128])
                    outt = ecmb.tile([128, 128], F32, tag="outt")
                    # un-permute (d,h) columns back to (h,d) via strided write
                    ot = outt[:]
                    otperm = bass.AP(ot.tensor, ot.offset,
                                     [ot.ap[0], [1, D_OUT], [D_OUT, HEADS]])
                    nc.gpsimd.tensor_scalar_max(otperm, acc[:], 0.0)
                    nc.sync.dma_start(out=out[w * 128:(w + 1) * 128, :],
                                      in_=outt[:])

    nc.finalize()
    return nc


def _fold(W, a):
    # W: [C_IN, HD] f32, a: [HEADS, D_OUT] -> [C_IN, HEADS]
    return np.einsum("chd,hd->ch",
                     W.astype(np.float64).reshape(C_IN, HEADS, D_OUT),
                     a.astype(np.float64)).astype(np.float32)


# (d,h)-interleaved column permutation: new col d*HEADS+h <- old col h*D_OUT+d
_PERM = np.array([h * D_OUT + d for d in range(D_OUT) for h in range(HEADS)],
                 dtype=np.int64)


def _edge_arrays(tgt, src, bph):
    """Per-core idx16 / tgtl / count arrays for one edge set."""
    bpw = 2 * bph
    slots_h = bph * 128
    iph = slots_h // 16
    idx_all = np.full((N_CORES, 128, NW * 2 * iph), -1, np.int16)
    tgl_all = np.full((N_CORES, 128, NW * bpw), -1.0, np.float32)
    cnt_all = np.zeros((N_CORES, 1, NW * 2), np.int32)
    order = np.argsort(tgt, kind="stable")
    tgt_s = tgt[order]
    src_s = src[order]
    core_of = tgt_s // CPC
    core_of = np.minimum(core_of, N_CORES - 1)
    for c in range(N_CORES):
        m = core_of == c
        tc_, sc_ = tgt_s[m] - c * CPC, src_s[m]
        wi = tc_ // 128
        tl = tc_ - wi * 128
        for w in range(NW):
            mw = wi == w
            tw, sw = tl[mw], sc_[mw]
            for half in range(2):
                if half == 0:
                    mh = sw < HALF
                    sidx = sw[mh]
                else:
                    mh = sw >= HALF
                    sidx = sw[mh] - HALF
                th = tw[mh]
                n = len(sidx)
                if n > slots_h:
                    raise OverflowError("half-window overflow")
                flat_i = np.full(slots_h, -1, np.int16)
                flat_i[:n] = sidx.astype(np.int16)
                wrap = flat_i.reshape(iph, 16).T  # [16, iph]
                col0 = (w * 2 + half) * iph
                idx_all[c, :, col0:col0 + iph] = np.tile(wrap, (8, 1))
                # tgtl: slot (b,p): block b within window = half*bph + i//128
                tl_flat = np.full(slots_h, -1.0, np.float32)
                tl_flat[:n] = th.astype(np.float32)
                blk = tl_flat.reshape(bph, 128)  # [b, p]
                b0 = w * bpw + half * bph
                tgl_all[c, :, b0:b0 + bph] = blk.T
                cnt_all[c, 0, w * 2 + half] = n
    return idx_all, tgl_all, cnt_all


def _max_half_count(tgt, src):
    mx = 0
    tgt = np.asarray(tgt)
    src = np.asarray(src)
    core_of = np.minimum(tgt // CPC, N_CORES - 1)
    for c in range(N_CORES):
        m = core_of == c
        tc_, sc_ = tgt[m] - c * CPC, src[m]
        wi = tc_ // 128
        for half in range(2):
            sel = (sc_ < HALF) if half == 0 else (sc_ >= HALF)
            if sel.any():
                cnt = np.bincount(wi[sel], minlength=NW)
                mx = max(mx, int(cnt.max()))
    return mx


def kernel(x, lower_tgt, lower_src, upper_tgt, upper_src,
           W_low, a_src_low, a_dst_low, W_up, a_src_up, a_dst_up,
           W_skip, b_skip):
    mx = max(_max_half_count(lower_tgt, lower_src),
             _max_half_count(upper_tgt, upper_src))
    bph = max(2, -(-mx // 128))
    if bph not in _CACHED:
        _CACHED[bph] = _build_nc(bph)
    nc = _CACHED[bph]

    x = np.asarray(x, np.float32)
    xT_full = np.zeros((C_IN, max(NPAD, N_CORES * CPC)), ml_dtypes.bfloat16)
    xT_full[:, :N_CELLS] = np.ascontiguousarray(
        x.astype(ml_dtypes.bfloat16).T)

    w_all = np.zeros((C_IN, 264), np.float32)
    w_all[:, 0:128] = W_low[:, _PERM]
    w_all[:, 128:132] = _fold(W_low, a_src_low)
    w_all[:, 132:260] = W_up[:, _PERM]
    w_all[:, 260:264] = _fold(W_up, a_src_up)
    w_all = w_all.astype(ml_dtypes.bfloat16)

    w_own = np.zeros((C_IN, 136), np.float32)
    w_own[:, 0:4] = _fold(W_low, a_dst_low)
    w_own[:, 4:8] = _fold(W_up, a_dst_up)
    w_own[:, 8:136] = EPS * W_skip[:, _PERM]
    w_own = w_own.astype(ml_dtypes.bfloat16)

    b_rep = np.broadcast_to((EPS * b_skip).astype(np.float32)[_PERM],
                            (128, 128)).copy()
    iota = np.broadcast_to(np.arange(128, dtype=ml_dtypes.bfloat16),
                           (128, 128)).copy()
    ident = np.eye(128, dtype=ml_dtypes.bfloat16)

    idx0, tgl0, cnt0 = _edge_arrays(np.asarray(lower_tgt),
                                    np.asarray(lower_src), bph)
    idx1, tgl1, cnt1 = _edge_arrays(np.asarray(upper_tgt),
                                    np.asarray(upper_src), bph)

    in_maps = []
    for c in range(N_CORES):
        xoT = np.ascontiguousarray(xT_full[:, c * CPC:(c + 1) * CPC])
        in_maps.append(dict(
            xT_bf=xT_full[:, :NPAD], xT_own=xoT, w_all=w_all, w_own=w_own,
            b_rep=b_rep,
            iota=iota, ident=ident,
            idx16_0=idx0[c], idx16_1=idx1[c], tgtl_0=tgl0[c], tgtl_1=tgl1[c],
            cnt_0=cnt0[c], cnt_1=cnt1[c],
        ))

    res = run_bass_kernel_spmd(nc, in_maps, core_ids=list(range(N_CORES)),
                               trace=TRACE)
    outs = []
    for c in range(N_CORES):
        lo = c * CPC
        hi = min(lo + CPC, N_CELLS)
        outs.append(res.results[c]["out"][:hi - lo])
    full = np.concatenate(outs, axis=0)
    if TRACE:
        kernel.last_exec_ns = res.exec_time_ns
        kernel.last_results = res
    return full.astype(np.float32)


# revision 47
# speedup vs baseline: 85.3691x; 1.0654x over previous
"""CANLayer (two-edge-set multi-head cell attention + skip) on 8 TRN2 NeuronCores.

Self-contained: hardcodes shapes for N=50000 cells, E=800000 edges/set,
C_IN=128, HEADS=4, D_OUT=32.

Strategy (v2):
 - Cells are 1D-partitioned across 8 cores (6272 aligned cells each); edges are
   routed to the core owning their target cell (host-side, part of sharding).
 - Each core redundantly computes a per-node table in DRAM (one merged table,
   1024B rows):
     row[n] = [xm_l_perm(128 bf16) | ss_l(4 bf16) | 0*124 |
               xm_u_perm(128 bf16) | ss_u(4 bf16) | 0*124]
   where xm = x @ W_s with (d,h)-interleaved columns and ss = x @ (W_s @ a_src)
   (attention source logit).  The node matmul writes bf16 straight to PSUM and
   one DMA per 128-cell tile copies PSUM -> DRAM (no staging copies).
 - Edge phase: per 128-target-cell window, dma_gather pulls 512B row halves for
   each edge (int16 indices, split over two table halves); attention weights
   use the shift-free identity  softmax(LR(ss+sd)) == normalize over segment of
   exp(LR(ss+sd)), with sd broadcast from the window's target cells via a
   one-hot^T matmul (transposes batched through PSUM in groups, single DVE
   copy); message weighting is one batched multiply per window (packed bf16
   APs via the (d,h) interleave); aggregation is a one-hot matmul accumulated
   in PSUM (cells x [128 msg | 4 denom]).
 - Output: relu(agg_low/denom_low + agg_up/denom_up + EPS*(x@W_skip+b_skip)),
   columns un-permuted during the final strided write.
"""
import sys
sys.path.insert(0, "/opt/trn_rl_repo")

import os

import numpy as np
import ml_dtypes

import concourse.bass as bass
import concourse.mybir as mybir
import concourse.tile as tile
from concourse import bacc
from concourse.bass_utils import run_bass_kernel_spmd

BF16 = mybir.dt.bfloat16
F32 = mybir.dt.float32
I16 = mybir.dt.int16

N_CELLS = 50000
N_EDGES = 800000
C_IN = 128
HEADS = 4
D_OUT = 32
HD = HEADS * D_OUT          # 128
EPS = 1.0 + 1e-6
NEG_SLOPE = 0.01

N_CORES = 8
CPC = 6272                  # cells per core (49 * 128), last core ragged
NW = 49                     # windows (128 cells) per core
NT = 391                    # node tiles over padded 50048 cells
NPAD = NT * 128             # 50048
TAB_ROWS = NPAD             # table rows
HALF = 25024                # int16-index table split
RROW = 512                  # merged table row (bf16 elems) = 1024B
RCOL = 256                  # per-set row half (bf16 elems) = 512B
XCOL = 128                  # xm cols
TRACE = False
NW_RUN = int(os.environ.get("KERNEL_NW", NW))
SIM_SAFE = os.environ.get("KERNEL_SIM_SAFE", "0") == "1"
STAGE = int(os.environ.get("KERNEL_STAGE", "3"))

_CACHED = {}


def _build_nc(bph):
    bpw = 2 * bph            # blocks (128 edges) per window
    slots_h = bph * 128      # slots per half per window
    iph = slots_h // 16      # idx16 cols per half

    nc = bacc.Bacc(None)

    # ---- per-core inputs ----
    xT_bf = nc.declare_dram_parameter("xT_bf", [C_IN, NPAD], BF16, isOutput=False)
    xT_own = nc.declare_dram_parameter("xT_own", [C_IN, CPC], BF16, isOutput=False)
    w_all = nc.declare_dram_parameter("w_all", [C_IN, 264], BF16, isOutput=False)
    w_own = nc.declare_dram_parameter("w_own", [C_IN, 136], BF16, isOutput=False)
    b_rep = nc.declare_dram_parameter("b_rep", [128, 128], F32, isOutput=False)
    iota_in = nc.declare_dram_parameter("iota", [128, 128], BF16, isOutput=False)
    ident_in = nc.declare_dram_parameter("ident", [128, 128], BF16, isOutput=False)
    idx16 = [
        nc.declare_dram_parameter(f"idx16_{s}", [128, NW * 2 * iph], I16,
                                  isOutput=False)
        for s in range(2)
    ]
    tgtl = [
        nc.declare_dram_parameter(f"tgtl_{s}", [128, NW * bpw], F32, isOutput=False)
        for s in range(2)
    ]
    cnts = [
        nc.declare_dram_parameter(f"cnt_{s}", [1, NW * 2], mybir.dt.int32,
                                  isOutput=False)
        for s in range(2)
    ]
    out = nc.declare_dram_parameter("out", [CPC, HD], F32, isOutput=True)

    # ---- DRAM internals ----
    mtable = nc.dram_tensor("mtable", [TAB_ROWS, RROW], BF16)

    with tile.TileContext(nc) as tc:
        # ---------- persistent SBUF ----------
        with tc.tile_pool(name="persist", bufs=1) as pers:
            t_iota = pers.tile([128, 128], BF16)
            t_ident = pers.tile([128, 128], BF16)
            t_brep = pers.tile([128, 128], F32)
            t_idx = [pers.tile([128, NW * 2 * iph], I16, tag=f"idx{s}", name=f"tidx{s}") for s in range(2)]
            t_tgtl = [pers.tile([128, NW * bpw], F32, tag=f"tgtl{s}", name=f"ttgtl{s}") for s in range(2)]
            t_sdw = [pers.tile([128, NW * HEADS], BF16, tag=f"sdw{s}", name=f"tsdw{s}") for s in range(2)]
            t_skip = pers.tile([128, NW * 128], F32)
            t_cnt = [pers.tile([1, NW * 2], mybir.dt.int32, tag=f"cnt{s}",
                               name=f"tcnt{s}") for s in range(2)]

            nc.sync.dma_start(out=t_iota[:], in_=iota_in[:])
            nc.sync.dma_start(out=t_ident[:], in_=ident_in[:])
            nc.sync.dma_start(out=t_brep[:], in_=b_rep[:])
            for s in range(2):
                nc.sync.dma_start(out=t_idx[s][:], in_=idx16[s][:])
                nc.sync.dma_start(out=t_tgtl[s][:], in_=tgtl[s][:])
                nc.sync.dma_start(out=t_cnt[s][:], in_=cnts[s][:])

            # ---------- node phase ----------
            with tc.tile_pool(name="node_sb", bufs=1) as nsb, \
                 tc.tile_pool(name="node_stage", bufs=3) as nst, \
                 tc.tile_pool(name="node_ps", bufs=4, space="PSUM") as nps:
                t_wall = nsb.tile([128, 264], BF16)
                t_wown = nsb.tile([128, 136], BF16)
                nc.sync.dma_start(out=t_wall[:], in_=w_all[:])
                nc.sync.dma_start(out=t_wown[:], in_=w_own[:])

                t_xT = nsb.tile([128, NPAD], BF16)
                nc.sync.dma_start(out=t_xT[:], in_=xT_bf[:])

                # ps = [row_l(132) | row_u(132)] f32; two staged copies build
                # the merged (zero-padded) 1024B row; one batched DMA per
                # TB tiles (HWDGE/SP-SEQ cost is per-instruction)
                TB = 8
                for t0 in range(0, NT, TB):
                    tn = min(TB, NT - t0)
                    stg = nst.tile([128, TB, RROW], BF16, tag="stg")
                    if SIM_SAFE or t0 < 3 * TB:
                        nc.gpsimd.memset(stg[:], 0)
                    for tt in range(tn):
                        t = t0 + tt
                        ps = nps.tile([128, 264], F32, tag="nps")
                        nc.tensor.matmul(ps[:], t_xT[:, t * 128:(t + 1) * 128],
                                         t_wall[:], start=True, stop=True)
                        nc.scalar.copy(out=stg[:, tt, 0:132], in_=ps[:, 0:132])
                        nc.vector.tensor_copy(out=stg[:, tt, RCOL:RCOL + 132],
                                              in_=ps[:, 132:264])
                    mt = mtable[:]
                    mrows = bass.AP(mt.tensor, t0 * 128 * RROW,
                                    [[RROW, 128], [128 * RROW, tn], [1, RROW]])
                    nc.sync.dma_start(out=mrows, in_=stg[:, 0:tn, :])

                # own pass: sd + skip for this core's cells
                t_xoT = nsb.tile([128, CPC], BF16)
                nc.sync.dma_start(out=t_xoT[:], in_=xT_own[:])
                for t in range(NW):
                    ps = nps.tile([128, 136], F32, tag="ops")
                    nc.tensor.matmul(ps[:], t_xoT[:, t * 128:(t + 1) * 128],
                                     t_wown[:], start=True, stop=True)
                    for s in range(2):
                        nc.vector.tensor_copy(
                            out=t_sdw[s][:, t * HEADS:(t + 1) * HEADS],
                            in_=ps[:, 4 * s:4 * s + 4])
                    # skip with bias
                    nc.vector.scalar_tensor_tensor(
                        out=t_skip[:, t * 128:(t + 1) * 128],
                        in0=ps[:, 8:136], scalar=0.0,
                        in1=t_brep[:],
                        op0=mybir.AluOpType.add, op1=mybir.AluOpType.add)

            # ---------- edge phase ----------
            GH = (bpw + 1) // 2  # transpose group size (blocks per PSUM group)
            with tc.tile_pool(name="eg", bufs=3) as egp, \
                 tc.tile_pool(name="ea", bufs=2) as eap, \
                 tc.tile_pool(name="esm", bufs=2) as esm, \
                 tc.tile_pool(name="eat", bufs=2) as eat, \
                 tc.tile_pool(name="eps", bufs=2, space="PSUM") as epp, \
                 tc.tile_pool(name="epsb", bufs=2, space="PSUM") as epb, \
                 tc.tile_pool(name="epst", bufs=2, space="PSUM") as ept, \
                 tc.tile_pool(name="ecmb", bufs=2) as ecmb:
                for w in range(NW_RUN):
                    ps_agg = None
                    if STAGE >= 3:
                        ps_agg = epp.tile([128, 2, 132], F32, tag="agg")
                    for s in range(2 if STAGE >= 1 else 0):
                        G = egp.tile([128, bpw, RCOL], BF16, tag="G")
                        if SIM_SAFE or w == 0:
                            nc.gpsimd.memset(G[:], 0)
                        for half in range(2):
                            nreg = nc.gpsimd.value_load(
                                t_cnt[s][0:1, w * 2 + half:w * 2 + half + 1])
                            nc.gpsimd.dma_gather(
                                out_ap=G[:, half * bph:(half + 1) * bph, :],
                                in_ap=mtable[half * HALF:half * HALF + HALF,
                                             s * RCOL:(s + 1) * RCOL],
                                idxs_ap=t_idx[s][:, (w * 2 + half) * iph:
                                                 (w * 2 + half + 1) * iph],
                                num_idxs=slots_h,
                                num_idxs_reg=nreg,
                                elem_size=RCOL,
                                elem_step=RROW,
                                single_packet=False,
                            )
                        if STAGE < 2:
                            continue
                        A = eap.tile([128, bpw, 128], BF16, tag="A")
                        sd_ps = epb.tile([128, bpw * HEADS], F32, tag="sdps")
                        for b in range(bpw):
                            nc.vector.tensor_scalar(
                                out=A[:, b, :], in0=t_iota[:],
                                scalar1=t_tgtl[s][:, w * bpw + b:w * bpw + b + 1],
                                scalar2=None, op0=mybir.AluOpType.is_equal)
                        # transpose A blocks through PSUM in two groups with a
                        # single Act copy per group
                        for g in range(2):
                            b0, b1 = g * GH, min((g + 1) * GH, bpw)
                            atp = ept.tile([128, GH, 128], BF16, tag="atp")
                            for j in range(b1 - b0):
                                nc.tensor.transpose(out=atp[:, j, :],
                                                    in_=A[:, b0 + j, :],
                                                    identity=t_ident[:])
                            at_sb = eat.tile([128, GH, 128], BF16, tag="atsb")
                            nc.scalar.copy(out=at_sb[:], in_=atp[:])
                            for j in range(b1 - b0):
                                b = b0 + j
                                nc.tensor.matmul(
                                    sd_ps[:, b * HEADS:(b + 1) * HEADS],
                                    at_sb[:, j, :],
                                    t_sdw[s][:, w * HEADS:(w + 1) * HEADS],
                                    start=True, stop=True)
                        # window-batched softmax weights (ss is bf16 in-row)
                        alpha = esm.tile([128, bpw * HEADS], F32, tag="alpha")
                        nc.vector.tensor_tensor(
                            out=alpha[:].rearrange("p (b h) -> p b h", h=HEADS),
                            in0=G[:, :, XCOL:XCOL + HEADS],
                            in1=sd_ps[:].rearrange("p (b h) -> p b h", h=HEADS),
                            op=mybir.AluOpType.add)
                        lr = esm.tile([128, bpw * HEADS], F32, tag="lr")
                        nc.vector.scalar_tensor_tensor(
                            out=lr[:], in0=alpha[:], scalar=NEG_SLOPE,
                            in1=alpha[:],
                            op0=mybir.AluOpType.mult, op1=mybir.AluOpType.max)
                        if STAGE < 3:
                            e_w = esm.tile([128, bpw * HEADS], BF16, tag="ew")
                            nc.scalar.activation(
                                out=e_w[:], in_=lr[:],
                                func=mybir.ActivationFunctionType.Exp)
                            continue
                        pme = egp.tile([128, bpw, 132], BF16, tag="pme")
                        # exp writes straight into pme's weight columns
                        nc.scalar.activation(
                            out=pme[:, :, 128:132],
                            in_=lr[:].rearrange("p (b h) -> p b h", h=HEADS),
                            func=mybir.ActivationFunctionType.Exp)
                        # one batched multiply for the whole window: xm columns
                        # are (d,h)-interleaved so every AP is packed bf16
                        pm = pme[:]
                        ew3 = bass.AP(pm.tensor, pm.offset + 128,
                                      [pm.ap[0], [132, bpw], [0, D_OUT],
                                       [1, HEADS]])
                        nc.vector.tensor_tensor(
                            out=pme[:, :, 0:XCOL], in0=G[:, :, 0:XCOL],
                            in1=ew3, op=mybir.AluOpType.mult)
                        for b in range(bpw):
                            nc.tensor.matmul(ps_agg[:, s, :], A[:, b, :],
                                             pme[:, b, :],
                                             start=(b == 0), stop=(b == bpw - 1))

                    # ---- combine window ----
                    if STAGE < 3:
                        outt0 = ecmb.tile([128, 128], F32, tag="outt")
                        nc.vector.tensor_scalar_max(
                            outt0[:], t_skip[:, w * 128:(w + 1) * 128], 0.0)
                        nc.sync.dma_start(out=out[w * 128:(w + 1) * 128, :],
                                          in_=outt0[:])
                        continue
                    # copy PSUM agg to SBUF once (Act), then combine on Pool
                    agg_sb = ecmb.tile([128, 2, 132], F32, tag="aggsb")
                    nc.scalar.copy(out=agg_sb[:], in_=ps_agg[:])
                    dn = ecmb.tile([128, 2 * HEADS], F32, tag="dn")
                    nc.vector.tensor_scalar_add(
                        dn[:].rearrange("p (s h) -> p s h", h=HEADS),
                        agg_sb[:, :, 128:132], 1e-16)
                    rc = ecmb.tile([128, 2 * HEADS], F32, tag="rc")
                    nc.vector.reciprocal(out=rc[:], in_=dn[:])
                    # both sets' weighted messages in one op
                    accb = ecmb.tile([128, 2, 128], F32, tag="accb")
                    rca = rc[:]
                    rcb = bass.AP(rca.tensor, rca.offset,
                                  [rca.ap[0], [HEADS, 2], [0, D_OUT],
                                   [1, HEADS]])
                    nc.gpsimd.tensor_tensor(
                        out=accb[:], in0=agg_sb[:, :, 0:128],
                        in1=rcb, op=mybir.AluOpType.mult)
                    acc = ecmb.tile([128, 128], F32, tag="acc")
                    nc.gpsimd.tensor_tensor(out=acc[:], in0=accb[:, 0, :],
                                            in1=accb[:, 1, :],
                                            op=mybir.AluOpType.add)
                    nc.gpsimd.tensor_add(out=acc[:], in0=acc[:],
                                         in1=t_skip[:, w * 128:(w + 1) * 128])
                    outt = ecmb.tile([128, 128], F32, tag="outt")
                    # un-permute (d,h) columns back to (h,d) via strided write
                    ot = outt[:]
                    otperm = bass.AP(ot.tensor, ot.offset,
                                     [ot.ap[0], [1, D_OUT], [D_OUT, HEADS]])
                    nc.gpsimd.tensor_scalar_max(otperm, acc[:], 0.0)
                    nc.sync.dma_start(out=out[w * 128:(w + 1) * 128, :],
                                      in_=outt[:])

    nc.finalize()
    return nc


def _fold(W, a):
    # W: [C_IN, HD] f32, a: [HEADS, D_OUT] -> [C_IN, HEADS]
    return np.einsum("chd,hd->ch",
                     W.astype(np.float64).reshape(C_IN, HEADS, D_OUT),
                     a.astype(np.float64)).astype(np.float32)


# (d,h)-interleaved column permutation: new col d*HEADS+h <- old col h*D_OUT+d
_PERM = np.array([h * D_OUT + d for d in range(D_OUT) for h in range(HEADS)],
                 dtype=np.int64)


def _edge_arrays(tgt, src, bph):
    """Per-core idx16 / tgtl / count arrays for one edge set."""
    bpw = 2 * bph
    slots_h = bph * 128
    iph = slots_h // 16
    idx_all = np.full((N_CORES, 128, NW * 2 * iph), -1, np.int16)
    tgl_all = np.full((N_CORES, 128, NW * bpw), -1.0, np.float32)
    cnt_all = np.zeros((N_CORES, 1, NW * 2), np.int32)
    order = np.argsort(tgt, kind="stable")
    tgt_s = tgt[order]
    src_s = src[order]
    core_of = tgt_s // CPC
    core_of = np.minimum(core_of, N_CORES - 1)
    for c in range(N_CORES):
        m = core_of == c
        tc_, sc_ = tgt_s[m] - c * CPC, src_s[m]
        wi = tc_ // 128
        tl = tc_ - wi * 128
        for w in range(NW):
            mw = wi == w
            tw, sw = tl[mw], sc_[mw]
            for half in range(2):
                if half == 0:
                    mh = sw < HALF
                    sidx = sw[mh]
                else:
                    mh = sw >= HALF
                    sidx = sw[mh] - HALF
                th = tw[mh]
                n = len(sidx)
                if n > slots_h:
                    raise OverflowError("half-window overflow")
                flat_i = np.full(slots_h, -1, np.int16)
                flat_i[:n] = sidx.astype(np.int16)
                wrap = flat_i.reshape(iph, 16).T  # [16, iph]
                col0 = (w * 2 + half) * iph
                idx_all[c, :, col0:col0 + iph] = np.tile(wrap, (8, 1))
                # tgtl: slot (b,p): block b within window = half*bph + i//128
                tl_flat = np.full(slots_h, -1.0, np.float32)
                tl_flat[:n] = th.astype(np.float32)
                blk = tl_flat.reshape(bph, 128)  # [b, p]
                b0 = w * bpw + half * bph
                tgl_all[c, :, b0:b0 + bph] = blk.T
                cnt_all[c, 0, w * 2 + half] = n
    return idx_all, tgl_all, cnt_all


def _max_half_count(tgt, src):
    mx = 0
    tgt = np.asarray(tgt)
    src = np.asarray(src)
    core_of = np.minimum(tgt // CPC, N_CORES - 1)
    for c in range(N_CORES):
        m = core_of == c
        tc_, sc_ = tgt[m] - c * CPC, src[m]
        wi = tc_ // 128
        for half in range(2):
            sel = (sc_ < HALF) if half == 0 else (sc_ >= HALF)
            if sel.any():
                cnt = np.bincount(wi[sel], minlength=NW)
                mx = max(mx, int(cnt.max()))
    return mx


def kernel(x, lower_tgt, lower_src, upper_tgt, upper_src,
           W_low, a_src_low, a_dst_low, W_up, a_src_up, a_dst_up,
           W_skip, b_skip):
    mx = max(_max_half_count(lower_tgt, lower_src),
             _max_half_count(upper_tgt, upper_src))
    bph = max(2, -(-mx // 128))
    if bph not in _CACHED:
        _CACHED[bph] = _build_nc(bph)
    nc = _CACHED[bph]

    x = np.asarray(x, np.float32)
    xT_full = np.zeros((C_IN, max(NPAD, N_CORES * CPC)), ml_dtypes.bfloat16)
    xT_full[:, :N_CELLS] = np.ascontiguousarray(
        x.astype(ml_dtypes.bfloat16).T)

    w_all = np.zeros((C_IN, 264), np.float32)
    w_all[:, 0:128] = W_low[:, _PERM]
    w_all[:, 128:132] = _fold(W_low, a_src_low)
    w_all[:, 132:260] = W_up[:, _PERM]
    w_all[:, 260:264] = _fold(W_up, a_src_up)
    w_all = w_all.astype(ml_dtypes.bfloat16)

    w_own = np.zeros((C_IN, 136), np.float32)
    w_own[:, 0:4] = _fold(W_low, a_dst_low)
    w_own[:, 4:8] = _fold(W_up, a_dst_up)
    w_own[:, 8:136] = EPS * W_skip[:, _PERM]
    w_own = w_own.astype(ml_dtypes.bfloat16)

    b_rep = np.broadcast_to((EPS * b_skip).astype(np.float32)[_PERM],
                            (128, 128)).copy()
    iota = np.broadcast_to(np.arange(128, dtype=ml_dtypes.bfloat16),
                           (128, 128)).copy()
    ident = np.eye(128, dtype=ml_dtypes.bfloat16)

    idx0, tgl0, cnt0 = _edge_arrays(np.asarray(lower_tgt),
                                    np.asarray(lower_src), bph)
    idx1, tgl1, cnt1 = _edge_arrays(np.asarray(upper_tgt),
                                    np.asarray(upper_src), bph)

    in_maps = []
    for c in range(N_CORES):
        xoT = np.ascontiguousarray(xT_full[:, c * CPC:(c + 1) * CPC])
        in_maps.append(dict(
            xT_bf=xT_full[:, :NPAD], xT_own=xoT, w_all=w_all, w_own=w_own,
            b_rep=b_rep,
            iota=iota, ident=ident,
            idx16_0=idx0[c], idx16_1=idx1[c], tgtl_0=tgl0[c], tgtl_1=tgl1[c],
            cnt_0=cnt0[c], cnt_1=cnt1[c],
        ))

    res = run_bass_kernel_spmd(nc, in_maps, core_ids=list(range(N_CORES)),
                               trace=TRACE)
    outs = []
    for c in range(N_CORES):
        lo = c * CPC
        hi = min(lo + CPC, N_CELLS)
        outs.append(res.results[c]["out"][:hi - lo])
    full = np.concatenate(outs, axis=0)
    if TRACE:
        kernel.last_exec_ns = res.exec_time_ns
        kernel.last_results = res
    return full.astype(np.float32)
